# revision 29
# baseline (speedup 1.0000x reference)
"""Bahdanau attention Trainium2 kernel.

Problem (per full input):
    query   [16, 1024] f32
    keys    [16, 8192, 256] f32
    values  [16, 8192, 256] f32
    kv_mask [16, 8192, 1] f32
    W_attr  [256, 1024], b_attr [256], V_attr [256] f32

    q = query @ W_attr.T                       # [B, K]
    e = (tanh(keys + q + b) @ V)               # [B, S]
    e = mask*e + (1-mask)*(-1e5)
    scores = softmax(e, axis=S)                # [B, S, 1]
    ctx = sum_s values * scores                # [B, K]

Strategy: data-parallel over batch across 8 cores (2 batches/core);
params replicated.  Inside a core, sequence dim tiled with the layout
s = p*64 + j (p = SBUF partition, j = free) so every DMA is contiguous
per partition.  Softmax is computed without max subtraction (energies
are bounded by sum|V| ~ 16, exp cannot overflow in f32); exp partial
sums ride on the activation instruction's accum_out, the final
normalizer is replicated across partitions with an all-ones matmul,
and ctx is accumulated on the tensor engine in PSUM.
"""

import numpy as np

import concourse.bass as bass
import concourse.bacc as bacc
import concourse.tile as tile
from concourse import mybir
from contextlib import ExitStack

F32 = mybir.dt.float32
AX = mybir.AluOpType
AF = mybir.ActivationFunctionType

P = 128
FLOAT_MIN = -100000.0


def build_bahdanau(BP=2, S=8192, K=256, Q=1024, JC=8, energy_mode="stt",
                   masked=False, add_mode="pe"):
    """Build the per-core Bass program.

    BP: batches per core.  JC: j-columns per streamed tile (tile free dim
    JC*K).  energy_mode:
      "mulred": tensor_mul by V then one big tensor_reduce  (2 DVE passes)
      "stt":    per-j fused scalar_tensor_tensor with accum_out (1 DVE pass)
    masked: emit the kv_mask arithmetic (exact form m*e + (1-m)*FLOAT_MIN).
      When the mask is all-ones (the common case) build with masked=False:
      the mask is then mathematically a no-op and is skipped entirely.
    """
    JTOT = S // P          # j columns per partition per batch
    NT = JTOT // JC        # streamed tiles per batch
    KH = K // P            # k partition-halves of W
    QC = Q // P            # q chunks of W/query

    nc = bacc.Bacc("TRN2", target_bir_lowering=False, debug=False)

    keys_d = nc.dram_tensor("keys", [BP, S, K], F32, kind="ExternalInput")
    values_d = nc.dram_tensor("values", [BP, S, K], F32, kind="ExternalInput")
    mask_d = nc.dram_tensor("mask", [BP, S], F32, kind="ExternalInput")
    query_d = nc.dram_tensor("query", [BP, Q], F32, kind="ExternalInput")
    w_d = nc.dram_tensor("w_attr", [K, Q], F32, kind="ExternalInput")
    b_d = nc.dram_tensor("b_attr", [K], F32, kind="ExternalInput")
    v_d = nc.dram_tensor("v_attr", [K], F32, kind="ExternalInput")

    scores_d = nc.dram_tensor("scores_out", [BP, S], F32, kind="ExternalOutput")
    ctx_d = nc.dram_tensor("ctx_out", [BP, K], F32, kind="ExternalOutput")

    ident_d = nc.inline_tensor(np.eye(P, dtype=np.float32), name="ident")
    ones_d = nc.inline_tensor(np.ones((P, P), dtype=np.float32), name="ones_pp")

    keys_v = keys_d[:].rearrange("b (p j) k -> b p j k", p=P)
    values_v = values_d[:].rearrange("b (p j) k -> b p j k", p=P)
    mask_v = mask_d[:].rearrange("b (p j) -> b p j", p=P)
    scores_v = scores_d[:].rearrange("b (p j) -> b p j", p=P)
    w_v = w_d[:].rearrange("(h p) q -> p h q", p=P)

    with tile.TileContext(nc) as tc, ExitStack() as ctx:
        consts = ctx.enter_context(tc.tile_pool(name="consts", bufs=1))
        prol = ctx.enter_context(tc.tile_pool(name="prol", bufs=1))
        kp = ctx.enter_context(tc.tile_pool(name="keysp", bufs=7))
        vp = ctx.enter_context(tc.tile_pool(name="valsp", bufs=7))
        ttp = ctx.enter_context(tc.tile_pool(name="tanhp", bufs=4))
        mp = ctx.enter_context(tc.tile_pool(name="maskp", bufs=2))
        epool = ctx.enter_context(tc.tile_pool(name="energyp", bufs=3))
        sp = ctx.enter_context(tc.tile_pool(name="scoresp", bufs=2))
        zp = ctx.enter_context(tc.tile_pool(name="zpartp", bufs=2))
        outp = ctx.enter_context(tc.tile_pool(name="outsp", bufs=2))
        scr = ctx.enter_context(tc.tile_pool(name="scratch", bufs=2))

        pst = ctx.enter_context(tc.tile_pool(name="pst", bufs=2, space="PSUM"))
        psc = ctx.enter_context(tc.tile_pool(name="psc", bufs=2, space="PSUM"))
        if add_mode == "pe":
            psa = ctx.enter_context(tc.tile_pool(name="psa", bufs=2, space="PSUM"))

        # ---- constants ----
        ident_sb = consts.tile([P, P], F32)
        nc.sync.dma_start(out=ident_sb, in_=ident_d[:])
        ones_sb = consts.tile([P, P], F32)
        nc.sync.dma_start(out=ones_sb, in_=ones_d[:])
        if add_mode == "pe":
            ident_r = consts.tile([P, P], mybir.dt.float32r)
            nc.sync.dma_start(
                out=ident_r, in_=ident_d[:].bitcast(mybir.dt.float32r)
            )
            ones_r = consts.tile([1, P], mybir.dt.float32r)
            nc.sync.dma_start(
                out=ones_r, in_=ones_d[0:1, :].bitcast(mybir.dt.float32r)
            )
        v_sb = consts.tile([P, K], F32)
        nc.sync.dma_start(out=v_sb, in_=v_d[:].partition_broadcast(P))
        b1_sb = consts.tile([1, K], F32)
        nc.sync.dma_start(out=b1_sb, in_=b_d[:].partition_broadcast(1))

        # ---- prologue: q projection  q = query @ W.T + b ----
        w_sb = prol.tile([P, KH, Q], F32)
        nc.sync.dma_start(out=w_sb, in_=w_v)
        q_sb = prol.tile([BP, Q], F32)
        nc.sync.dma_start(out=q_sb, in_=query_d[:])

        # transpose W -> Wt [q-part, k-free], and query -> qT [q-part, b]
        wt_sb = prol.tile([P, QC, K], F32)
        qt_sb = prol.tile([P, QC, BP], F32)
        for c in range(QC):
            for h in range(KH):
                tp = pst.tile([P, P], F32, tag="tp")
                nc.tensor.transpose(
                    tp[:, :], w_sb[:, h, c * P:(c + 1) * P], ident_sb[:, :]
                )
                nc.scalar.copy(wt_sb[:, c, h * P:(h + 1) * P], tp[:, :])
            tq = pst.tile([P, P], F32, tag="tp")
            nc.tensor.transpose(
                tq[:, 0:BP], q_sb[:, c * P:(c + 1) * P], ident_sb[0:BP, 0:BP]
            )
            nc.scalar.copy(qt_sb[:, c, :], tq[:, 0:BP])

        # per-batch projection row (each lands at partition 0)
        qbc = []      # dve add-mode: qb broadcast [P, K] per batch
        qbrow_r = []  # pe add-mode: qb row repeated JC times, float32r
        for b in range(BP):
            qp_ps = pst.tile([P, 2 * K], F32, tag="tp")
            for c in range(QC):
                nc.tensor.matmul(
                    qp_ps[0:1, 0:K], qt_sb[:, c, b:b + 1], wt_sb[:, c, :],
                    start=(c == 0), stop=(c == QC - 1),
                )
            qb_row = prol.tile([1, K], F32, tag=f"qbrow{b}")
            nc.vector.tensor_add(qb_row[:, :], qp_ps[0:1, 0:K], b1_sb[:, :])

            if add_mode == "pe":
                qr = prol.tile([1, JC * K], mybir.dt.float32r, tag=f"qbr{b}")
                nc.vector.tensor_copy(
                    qr[:, :].rearrange("p (j k) -> p j k", j=JC),
                    qb_row[:, :]
                    .rearrange("p (j k) -> p j k", j=1)
                    .to_broadcast([1, JC, K]),
                )
                qbrow_r.append(qr)
            else:
                qbb = pst.tile([P, 2 * K], F32, tag="tp")
                nc.tensor.matmul(
                    qbb[:, 0:K], ones_sb[0:1, :], qb_row[:, :],
                    start=True, stop=True,
                )
                qb_b = prol.tile([P, K], F32, tag=f"qbc{b}")
                nc.scalar.copy(qb_b[:, :], qbb[:, 0:K])
                qbc.append(qb_b)

        # ---- main loop ----
        for b in range(BP):
            scores_sb = sp.tile([P, JTOT], F32, tag="scores")
            zpart = zp.tile([P, NT], F32, tag="zpart")
            ctx_ps = psc.tile([1, K], F32, tag="ctx")

            if add_mode != "pe":
                qb_bcast = (
                    qbc[b][:, :]
                    .rearrange("p (j k) -> p j k", j=1)
                    .to_broadcast([P, JC, K])
                )
            v_bcast = (
                v_sb[:, :]
                .rearrange("p (j k) -> p j k", j=1)
                .to_broadcast([P, JC, K])
            )

            for ji in range(NT):
                kdt = mybir.dt.float32r if add_mode == "pe" else F32
                kt = kp.tile([P, JC * K], kdt, tag="kt")
                nc.sync.dma_start(
                    out=kt,
                    in_=keys_v[b, :, ji * JC:(ji + 1) * JC, :].bitcast(kdt),
                )
                vt = vp.tile([P, JC, K], mybir.dt.float32r, tag="vt")
                nc.sync.dma_start(
                    out=vt,
                    in_=values_v[b, :, ji * JC:(ji + 1) * JC, :].bitcast(
                        mybir.dt.float32r
                    ),
                )
                if masked:
                    mt = mp.tile([P, JC], F32, tag="mt")
                    nc.sync.dma_start(
                        out=mt, in_=mask_v[b, :, ji * JC:(ji + 1) * JC]
                    )

                if add_mode == "pe":
                    # tt = tanh(kt + q + b): broadcast-add on PE (fp32r,
                    # ~1e-6 rounding), tanh reads PSUM.  High priority so
                    # adds/tanh sort ahead of earlier tiles' ctx matmuls in
                    # the in-order PE/ACT streams (ctx can drain late; the
                    # add chain gates everything downstream).
                    tt = ttp.tile([P, JC * K], F32, tag="tt")
                    HKE = 1024  # psum chunk (2 banks)
                    with tc.high_priority(offset=26):
                        for h in range(JC * K // HKE):
                            ps = psa.tile([P, HKE], F32, tag="addps")
                            for c in range(0, HKE, 512):
                                lo = h * HKE + c
                                nc.tensor.matmul(
                                    ps[:, c:c + 512], ones_r[:, :],
                                    qbrow_r[b][:, lo:lo + 512],
                                    start=True, stop=False,
                                )
                                nc.tensor.matmul(
                                    ps[:, c:c + 512], ident_r[:, :],
                                    kt[:, lo:lo + 512],
                                    start=False, stop=True,
                                )
                            nc.scalar.activation(
                                tt[:, h * HKE:(h + 1) * HKE], ps[:, :], AF.Tanh
                            )
                else:
                    tt = kt
                    with tc.high_priority(offset=24):
                        nc.vector.tensor_add(
                            kt[:, :].rearrange("p (j k) -> p j k", j=JC),
                            kt[:, :].rearrange("p (j k) -> p j k", j=JC),
                            qb_bcast,
                        )
                        nc.scalar.activation(kt[:, :], kt[:, :], AF.Tanh)

                # energies e[p, j] = sum_k tt * V
                et = epool.tile([P, JC], F32, tag="et")
                if energy_mode == "mulred":
                    mm = scr.tile([P, JC, K], F32, tag="mm")
                    nc.vector.tensor_mul(
                        mm[:, :, :],
                        tt[:, :].rearrange("p (j k) -> p j k", j=JC),
                        v_bcast,
                    )
                    nc.vector.tensor_reduce(
                        et[:, :], mm[:, :, :], axis=mybir.AxisListType.X, op=AX.add
                    )
                else:
                    junk = scr.tile([P, K], F32, tag="junk")
                    for jj in range(JC):
                        nc.vector.scalar_tensor_tensor(
                            out=junk[:, :],
                            in0=tt[:, jj * K:(jj + 1) * K],
                            scalar=1.0,
                            in1=v_sb[:, :],
                            op0=AX.mult,
                            op1=AX.mult,
                            accum_out=et[:, jj:jj + 1],
                        )

                if masked:
                    # exact masked energy: e' = m*e + (1-m)*FLOAT_MIN
                    # om = m*(-FLOAT_MIN) + FLOAT_MIN  (== (1-m)*FLOAT_MIN, 0 at m=1)
                    om = mp.tile([P, JC], F32, tag="om")
                    nc.vector.tensor_scalar(
                        out=om[:, :], in0=mt[:, :],
                        scalar1=-FLOAT_MIN, scalar2=FLOAT_MIN,
                        op0=AX.mult, op1=AX.add,
                    )
                    nc.vector.tensor_mul(et[:, :], et[:, :], mt[:, :])
                    nc.vector.tensor_add(et[:, :], et[:, :], om[:, :])

                # p = exp(e), accumulate partial Z per partition
                nc.scalar.activation(
                    scores_sb[:, ji * JC:(ji + 1) * JC], et[:, :], AF.Exp,
                    bias=0.0, scale=1.0,
                    accum_out=zpart[:, ji:ji + 1],
                )
                # fp32r copy of the exp block for the PE (keeps scores fp32)
                pcol = scr.tile([P, JC], mybir.dt.float32r, tag="pcol")
                nc.vector.tensor_copy(
                    pcol[:, :], scores_sb[:, ji * JC:(ji + 1) * JC]
                )

                # ctx += p.T @ values  (float32r: 4x faster PE)
                for jj in range(JC):
                    col = ji * JC + jj
                    nc.tensor.matmul(
                        ctx_ps[:, :],
                        pcol[:, jj:jj + 1],
                        vt[:, jj, :],
                        start=(col == 0),
                        stop=(col == JTOT - 1),
                    )

            # normalizer Z = sum over partitions+tiles, replicated to all
            # partitions.  High priority: these are tiny ops that the next
            # batch's work floods out of the engine streams otherwise,
            # delaying the output DMAs by tens of us.
            with tc.high_priority():
                zc = zp.tile([P, 1], F32, tag="zc")
                nc.vector.tensor_reduce(
                    zc[:, :], zpart[:, :], axis=mybir.AxisListType.X, op=AX.add
                )
                zrep = pst.tile([P, P], F32, tag="tp")
                nc.tensor.matmul(zrep[:, 0:1], ones_sb[:, :], zc[:, :],
                                 start=True, stop=True)
                rz = zp.tile([P, 1], F32, tag="rz")
                nc.vector.reciprocal(rz[:, :], zrep[:, 0:1])

                # normalize + store.  Output DMAs go through the Pool
                # sequencer: on the in-order SP stream they would stall the
                # next batch's input loads behind this batch's epilogue.
                nc.vector.tensor_scalar_mul(
                    scores_sb[:, :], scores_sb[:, :], rz[:, :]
                )
                nc.gpsimd.dma_start(out=scores_v[b], in_=scores_sb[:, :])

                cs = outp.tile([1, K], F32, tag="cs")
                nc.vector.tensor_scalar_mul(cs[:, :], ctx_ps[:, :], rz[0:1, 0:1])
                nc.gpsimd.dma_start(out=ctx_d[b:b + 1, :], in_=cs[:, :])

    nc.compile()
    return nc


_CACHE = {}


def _get_nc(masked):
    key = ("nc", masked)
    if key not in _CACHE:
        _CACHE[key] = build_bahdanau(masked=masked)
    return _CACHE[key]


def kernel(**inputs):
    from concourse.bass_utils import run_bass_kernel_spmd

    query = np.ascontiguousarray(np.asarray(inputs["query"], dtype=np.float32))
    keys = np.ascontiguousarray(np.asarray(inputs["keys"], dtype=np.float32))
    values = np.ascontiguousarray(np.asarray(inputs["values"], dtype=np.float32))
    kv_mask = np.ascontiguousarray(
        np.asarray(inputs["kv_mask"], dtype=np.float32)[:, :, 0]
    )
    w = np.ascontiguousarray(np.asarray(inputs["W_attr"], dtype=np.float32))
    bb = np.ascontiguousarray(np.asarray(inputs["b_attr"], dtype=np.float32))
    vv = np.ascontiguousarray(np.asarray(inputs["V_attr"], dtype=np.float32))

    B = keys.shape[0]
    NCORES = 8
    BP = B // NCORES

    masked = not bool(np.all(kv_mask == 1.0))
    nc = _get_nc(masked)
    in_maps = []
    for i in range(NCORES):
        sl = slice(i * BP, (i + 1) * BP)
        in_maps.append({
            "keys": np.ascontiguousarray(keys[sl]),
            "values": np.ascontiguousarray(values[sl]),
            "mask": np.ascontiguousarray(kv_mask[sl]),
            "query": np.ascontiguousarray(query[sl]),
            "w_attr": w,
            "b_attr": bb,
            "v_attr": vv,
        })

    res = run_bass_kernel_spmd(nc, in_maps, core_ids=list(range(NCORES)))
    scores = np.concatenate([r["scores_out"] for r in res.results], axis=0)
    ctxv = np.concatenate([r["ctx_out"] for r in res.results], axis=0)
    return scores[:, :, None].astype(np.float32), ctxv.astype(np.float32)


# revision 32
# speedup vs baseline: 4.1196x; 4.1196x over previous
"""Bahdanau attention Trainium2 kernel.

Problem (per full input):
    query   [16, 1024] f32
    keys    [16, 8192, 256] f32
    values  [16, 8192, 256] f32
    kv_mask [16, 8192, 1] f32
    W_attr  [256, 1024], b_attr [256], V_attr [256] f32

    q = query @ W_attr.T                       # [B, K]
    e = (tanh(keys + q + b) @ V)               # [B, S]
    e = mask*e + (1-mask)*(-1e5)
    scores = softmax(e, axis=S)                # [B, S, 1]
    ctx = sum_s values * scores                # [B, K]

Strategy: data-parallel over batch across 8 cores (2 batches/core);
params replicated.  Inside a core, sequence dim tiled with the layout
s = p*64 + j (p = SBUF partition, j = free) so every DMA is contiguous
per partition.  Softmax is computed without max subtraction (energies
are bounded by sum|V| ~ 16, exp cannot overflow in f32); exp partial
sums ride on the activation instruction's accum_out, the final
normalizer is replicated across partitions with an all-ones matmul,
and ctx is accumulated on the tensor engine in PSUM.
"""

import numpy as np

import concourse.bass as bass
import concourse.bacc as bacc
import concourse.tile as tile
from concourse import mybir
from contextlib import ExitStack

F32 = mybir.dt.float32
AX = mybir.AluOpType
AF = mybir.ActivationFunctionType

P = 128
FLOAT_MIN = -100000.0


def build_bahdanau(BP=2, S=8192, K=256, Q=1024, JC=8, energy_mode="stt",
                   masked=False, add_mode="pe", repeat=1):
    """Build the per-core Bass program.

    BP: batches per core.  JC: j-columns per streamed tile (tile free dim
    JC*K).  energy_mode:
      "mulred": tensor_mul by V then one big tensor_reduce  (2 DVE passes)
      "stt":    per-j fused scalar_tensor_tensor with accum_out (1 DVE pass)
    masked: emit the kv_mask arithmetic (exact form m*e + (1-m)*FLOAT_MIN).
      When the mask is all-ones (the common case) build with masked=False:
      the mask is then mathematically a no-op and is skipped entirely.
    """
    JTOT = S // P          # j columns per partition per batch
    NT = JTOT // JC        # streamed tiles per batch
    KH = K // P            # k partition-halves of W
    QC = Q // P            # q chunks of W/query

    nc = bacc.Bacc("TRN2", target_bir_lowering=False, debug=False)

    keys_d = nc.dram_tensor("keys", [BP, S, K], F32, kind="ExternalInput")
    values_d = nc.dram_tensor("values", [BP, S, K], F32, kind="ExternalInput")
    mask_d = nc.dram_tensor("mask", [BP, S], F32, kind="ExternalInput")
    query_d = nc.dram_tensor("query", [BP, Q], F32, kind="ExternalInput")
    w_d = nc.dram_tensor("w_attr", [K, Q], F32, kind="ExternalInput")
    b_d = nc.dram_tensor("b_attr", [K], F32, kind="ExternalInput")
    v_d = nc.dram_tensor("v_attr", [K], F32, kind="ExternalInput")

    scores_d = nc.dram_tensor("scores_out", [BP, S], F32, kind="ExternalOutput")
    ctx_d = nc.dram_tensor("ctx_out", [BP, K], F32, kind="ExternalOutput")

    ident_d = nc.inline_tensor(np.eye(P, dtype=np.float32), name="ident")
    ones_d = nc.inline_tensor(np.ones((P, P), dtype=np.float32), name="ones_pp")

    keys_v = keys_d[:].rearrange("b (p j) k -> b p j k", p=P)
    values_v = values_d[:].rearrange("b (p j) k -> b p j k", p=P)
    mask_v = mask_d[:].rearrange("b (p j) -> b p j", p=P)
    scores_v = scores_d[:].rearrange("b (p j) -> b p j", p=P)
    w_v = w_d[:].rearrange("(h p) q -> p h q", p=P)

    with tile.TileContext(nc) as tc, ExitStack() as ctx:
        consts = ctx.enter_context(tc.tile_pool(name="consts", bufs=1))
        prol = ctx.enter_context(tc.tile_pool(name="prol", bufs=1))
        kp = ctx.enter_context(tc.tile_pool(name="keysp", bufs=7))
        vp = ctx.enter_context(tc.tile_pool(name="valsp", bufs=7))
        ttp = ctx.enter_context(tc.tile_pool(name="tanhp", bufs=4))
        mp = ctx.enter_context(tc.tile_pool(name="maskp", bufs=2))
        epool = ctx.enter_context(tc.tile_pool(name="energyp", bufs=3))
        sp = ctx.enter_context(tc.tile_pool(name="scoresp", bufs=2))
        zp = ctx.enter_context(tc.tile_pool(name="zpartp", bufs=2))
        outp = ctx.enter_context(tc.tile_pool(name="outsp", bufs=2))
        scr = ctx.enter_context(tc.tile_pool(name="scratch", bufs=2))

        pst = ctx.enter_context(tc.tile_pool(name="pst", bufs=2, space="PSUM"))
        psc = ctx.enter_context(tc.tile_pool(name="psc", bufs=2, space="PSUM"))
        if add_mode == "pe":
            psa = ctx.enter_context(tc.tile_pool(name="psa", bufs=2, space="PSUM"))

        # ---- constants ----
        ident_sb = consts.tile([P, P], F32)
        nc.sync.dma_start(out=ident_sb, in_=ident_d[:])
        ones_sb = consts.tile([P, P], F32)
        nc.sync.dma_start(out=ones_sb, in_=ones_d[:])
        if add_mode == "pe":
            ident_r = consts.tile([P, P], mybir.dt.float32r)
            nc.sync.dma_start(
                out=ident_r, in_=ident_d[:].bitcast(mybir.dt.float32r)
            )
            ones_r = consts.tile([1, P], mybir.dt.float32r)
            nc.sync.dma_start(
                out=ones_r, in_=ones_d[0:1, :].bitcast(mybir.dt.float32r)
            )
        v_sb = consts.tile([P, K], F32)
        nc.sync.dma_start(out=v_sb, in_=v_d[:].partition_broadcast(P))
        b1_sb = consts.tile([1, K], F32)
        nc.sync.dma_start(out=b1_sb, in_=b_d[:].partition_broadcast(1))

        # ---- prologue: q projection  q = query @ W.T + b ----
        w_sb = prol.tile([P, KH, Q], F32)
        nc.sync.dma_start(out=w_sb, in_=w_v)
        q_sb = prol.tile([BP, Q], F32)
        nc.sync.dma_start(out=q_sb, in_=query_d[:])

        # transpose W -> Wt [q-part, k-free], and query -> qT [q-part, b]
        wt_sb = prol.tile([P, QC, K], F32)
        qt_sb = prol.tile([P, QC, BP], F32)
        for c in range(QC):
            for h in range(KH):
                tp = pst.tile([P, P], F32, tag="tp")
                nc.tensor.transpose(
                    tp[:, :], w_sb[:, h, c * P:(c + 1) * P], ident_sb[:, :]
                )
                nc.scalar.copy(wt_sb[:, c, h * P:(h + 1) * P], tp[:, :])
            tq = pst.tile([P, P], F32, tag="tp")
            nc.tensor.transpose(
                tq[:, 0:BP], q_sb[:, c * P:(c + 1) * P], ident_sb[0:BP, 0:BP]
            )
            nc.scalar.copy(qt_sb[:, c, :], tq[:, 0:BP])

        # per-batch projection row (each lands at partition 0)
        qbc = []      # dve add-mode: qb broadcast [P, K] per batch
        qbrow_r = []  # pe add-mode: qb row repeated JC times, float32r
        for b in range(BP):
            qp_ps = pst.tile([P, 2 * K], F32, tag="tp")
            for c in range(QC):
                nc.tensor.matmul(
                    qp_ps[0:1, 0:K], qt_sb[:, c, b:b + 1], wt_sb[:, c, :],
                    start=(c == 0), stop=(c == QC - 1),
                )
            qb_row = prol.tile([1, K], F32, tag=f"qbrow{b}")
            nc.vector.tensor_add(qb_row[:, :], qp_ps[0:1, 0:K], b1_sb[:, :])

            if add_mode == "pe":
                qr = prol.tile([1, JC * K], mybir.dt.float32r, tag=f"qbr{b}")
                nc.vector.tensor_copy(
                    qr[:, :].rearrange("p (j k) -> p j k", j=JC),
                    qb_row[:, :]
                    .rearrange("p (j k) -> p j k", j=1)
                    .to_broadcast([1, JC, K]),
                )
                qbrow_r.append(qr)
            else:
                qbb = pst.tile([P, 2 * K], F32, tag="tp")
                nc.tensor.matmul(
                    qbb[:, 0:K], ones_sb[0:1, :], qb_row[:, :],
                    start=True, stop=True,
                )
                qb_b = prol.tile([P, K], F32, tag=f"qbc{b}")
                nc.scalar.copy(qb_b[:, :], qbb[:, 0:K])
                qbc.append(qb_b)

        # ---- main loop (optionally repeated on-device for benchmarking) ----
        rep_cm = (
            tc.For_i(0, repeat, 1,
                     hint_engines=(mybir.EngineType.PE, mybir.EngineType.DVE,
                                   mybir.EngineType.Activation,
                                   mybir.EngineType.SP))
            if repeat > 1 else None
        )
        if rep_cm is not None:
            rep_cm.__enter__()
        for b in range(BP):
            scores_sb = sp.tile([P, JTOT], F32, tag="scores")
            zpart = zp.tile([P, NT], F32, tag="zpart")
            ctx_ps = psc.tile([1, K], F32, tag="ctx")

            if add_mode != "pe":
                qb_bcast = (
                    qbc[b][:, :]
                    .rearrange("p (j k) -> p j k", j=1)
                    .to_broadcast([P, JC, K])
                )
            v_bcast = (
                v_sb[:, :]
                .rearrange("p (j k) -> p j k", j=1)
                .to_broadcast([P, JC, K])
            )

            for ji in range(NT):
                kdt = mybir.dt.float32r if add_mode == "pe" else F32
                kt = kp.tile([P, JC * K], kdt, tag="kt")
                nc.sync.dma_start(
                    out=kt,
                    in_=keys_v[b, :, ji * JC:(ji + 1) * JC, :].bitcast(kdt),
                )
                vt = vp.tile([P, JC, K], mybir.dt.float32r, tag="vt")
                nc.sync.dma_start(
                    out=vt,
                    in_=values_v[b, :, ji * JC:(ji + 1) * JC, :].bitcast(
                        mybir.dt.float32r
                    ),
                )
                if masked:
                    mt = mp.tile([P, JC], F32, tag="mt")
                    nc.sync.dma_start(
                        out=mt, in_=mask_v[b, :, ji * JC:(ji + 1) * JC]
                    )

                if add_mode == "pe":
                    # tt = tanh(kt + q + b): broadcast-add on PE (fp32r,
                    # ~1e-6 rounding), tanh reads PSUM.  High priority so
                    # adds/tanh sort ahead of earlier tiles' ctx matmuls in
                    # the in-order PE/ACT streams (ctx can drain late; the
                    # add chain gates everything downstream).
                    tt = ttp.tile([P, JC * K], F32, tag="tt")
                    HKE = 1024  # psum chunk (2 banks)
                    with tc.high_priority(offset=26):
                        for h in range(JC * K // HKE):
                            ps = psa.tile([P, HKE], F32, tag="addps")
                            for c in range(0, HKE, 512):
                                lo = h * HKE + c
                                nc.tensor.matmul(
                                    ps[:, c:c + 512], ones_r[:, :],
                                    qbrow_r[b][:, lo:lo + 512],
                                    start=True, stop=False,
                                )
                                nc.tensor.matmul(
                                    ps[:, c:c + 512], ident_r[:, :],
                                    kt[:, lo:lo + 512],
                                    start=False, stop=True,
                                )
                            nc.scalar.activation(
                                tt[:, h * HKE:(h + 1) * HKE], ps[:, :], AF.Tanh
                            )
                else:
                    tt = kt
                    with tc.high_priority(offset=24):
                        nc.vector.tensor_add(
                            kt[:, :].rearrange("p (j k) -> p j k", j=JC),
                            kt[:, :].rearrange("p (j k) -> p j k", j=JC),
                            qb_bcast,
                        )
                        nc.scalar.activation(kt[:, :], kt[:, :], AF.Tanh)

                # energies e[p, j] = sum_k tt * V
                et = epool.tile([P, JC], F32, tag="et")
                if energy_mode == "mulred":
                    mm = scr.tile([P, JC, K], F32, tag="mm")
                    nc.vector.tensor_mul(
                        mm[:, :, :],
                        tt[:, :].rearrange("p (j k) -> p j k", j=JC),
                        v_bcast,
                    )
                    nc.vector.tensor_reduce(
                        et[:, :], mm[:, :, :], axis=mybir.AxisListType.X, op=AX.add
                    )
                else:
                    junk = scr.tile([P, K], F32, tag="junk")
                    for jj in range(JC):
                        nc.vector.scalar_tensor_tensor(
                            out=junk[:, :],
                            in0=tt[:, jj * K:(jj + 1) * K],
                            scalar=1.0,
                            in1=v_sb[:, :],
                            op0=AX.mult,
                            op1=AX.mult,
                            accum_out=et[:, jj:jj + 1],
                        )

                if masked:
                    # exact masked energy: e' = m*e + (1-m)*FLOAT_MIN
                    # om = m*(-FLOAT_MIN) + FLOAT_MIN  (== (1-m)*FLOAT_MIN, 0 at m=1)
                    om = mp.tile([P, JC], F32, tag="om")
                    nc.vector.tensor_scalar(
                        out=om[:, :], in0=mt[:, :],
                        scalar1=-FLOAT_MIN, scalar2=FLOAT_MIN,
                        op0=AX.mult, op1=AX.add,
                    )
                    nc.vector.tensor_mul(et[:, :], et[:, :], mt[:, :])
                    nc.vector.tensor_add(et[:, :], et[:, :], om[:, :])

                # p = exp(e), accumulate partial Z per partition
                nc.scalar.activation(
                    scores_sb[:, ji * JC:(ji + 1) * JC], et[:, :], AF.Exp,
                    bias=0.0, scale=1.0,
                    accum_out=zpart[:, ji:ji + 1],
                )
                # fp32r copy of the exp block for the PE (keeps scores fp32)
                pcol = scr.tile([P, JC], mybir.dt.float32r, tag="pcol")
                nc.vector.tensor_copy(
                    pcol[:, :], scores_sb[:, ji * JC:(ji + 1) * JC]
                )

                # ctx += p.T @ values  (float32r: 4x faster PE)
                for jj in range(JC):
                    col = ji * JC + jj
                    nc.tensor.matmul(
                        ctx_ps[:, :],
                        pcol[:, jj:jj + 1],
                        vt[:, jj, :],
                        start=(col == 0),
                        stop=(col == JTOT - 1),
                    )

            # normalizer Z = sum over partitions+tiles, replicated to all
            # partitions.  High priority: these are tiny ops that the next
            # batch's work floods out of the engine streams otherwise,
            # delaying the output DMAs by tens of us.
            with tc.high_priority():
                zc = zp.tile([P, 1], F32, tag="zc")
                nc.vector.tensor_reduce(
                    zc[:, :], zpart[:, :], axis=mybir.AxisListType.X, op=AX.add
                )
                zrep = pst.tile([P, P], F32, tag="tp")
                nc.tensor.matmul(zrep[:, 0:1], ones_sb[:, :], zc[:, :],
                                 start=True, stop=True)
                rz = zp.tile([P, 1], F32, tag="rz")
                nc.vector.reciprocal(rz[:, :], zrep[:, 0:1])

                # normalize + store.  Output DMAs go through the Pool
                # sequencer: on the in-order SP stream they would stall the
                # next batch's input loads behind this batch's epilogue.
                nc.vector.tensor_scalar_mul(
                    scores_sb[:, :], scores_sb[:, :], rz[:, :]
                )
                nc.gpsimd.dma_start(out=scores_v[b], in_=scores_sb[:, :])

                cs = outp.tile([1, K], F32, tag="cs")
                nc.vector.tensor_scalar_mul(cs[:, :], ctx_ps[:, :], rz[0:1, 0:1])
                nc.gpsimd.dma_start(out=ctx_d[b:b + 1, :], in_=cs[:, :])

        if rep_cm is not None:
            rep_cm.__exit__(None, None, None)

    nc.compile()
    return nc


_CACHE = {}


def _get_nc(masked):
    key = ("nc", masked)
    if key not in _CACHE:
        _CACHE[key] = build_bahdanau(masked=masked)
    return _CACHE[key]


def kernel(**inputs):
    from concourse.bass_utils import run_bass_kernel_spmd

    query = np.ascontiguousarray(np.asarray(inputs["query"], dtype=np.float32))
    keys = np.ascontiguousarray(np.asarray(inputs["keys"], dtype=np.float32))
    values = np.ascontiguousarray(np.asarray(inputs["values"], dtype=np.float32))
    kv_mask = np.ascontiguousarray(
        np.asarray(inputs["kv_mask"], dtype=np.float32)[:, :, 0]
    )
    w = np.ascontiguousarray(np.asarray(inputs["W_attr"], dtype=np.float32))
    bb = np.ascontiguousarray(np.asarray(inputs["b_attr"], dtype=np.float32))
    vv = np.ascontiguousarray(np.asarray(inputs["V_attr"], dtype=np.float32))

    B = keys.shape[0]
    NCORES = 8
    BP = B // NCORES

    masked = not bool(np.all(kv_mask == 1.0))
    nc = _get_nc(masked)
    in_maps = []
    for i in range(NCORES):
        sl = slice(i * BP, (i + 1) * BP)
        in_maps.append({
            "keys": np.ascontiguousarray(keys[sl]),
            "values": np.ascontiguousarray(values[sl]),
            "mask": np.ascontiguousarray(kv_mask[sl]),
            "query": np.ascontiguousarray(query[sl]),
            "w_attr": w,
            "b_attr": bb,
            "v_attr": vv,
        })

    res = run_bass_kernel_spmd(nc, in_maps, core_ids=list(range(NCORES)))
    scores = np.concatenate([r["scores_out"] for r in res.results], axis=0)
    ctxv = np.concatenate([r["ctx_out"] for r in res.results], axis=0)
    return scores[:, :, None].astype(np.float32), ctxv.astype(np.float32)


# revision 41
# speedup vs baseline: 4.6502x; 1.1288x over previous
"""Bahdanau attention Trainium2 kernel.

Problem (per full input):
    query   [16, 1024] f32
    keys    [16, 8192, 256] f32
    values  [16, 8192, 256] f32
    kv_mask [16, 8192, 1] f32
    W_attr  [256, 1024], b_attr [256], V_attr [256] f32

    q = query @ W_attr.T                       # [B, K]
    e = (tanh(keys + q + b) @ V)               # [B, S]
    e = mask*e + (1-mask)*(-1e5)
    scores = softmax(e, axis=S)                # [B, S, 1]
    ctx = sum_s values * scores                # [B, K]

Strategy: data-parallel over batch across 8 cores (2 batches/core);
params replicated.  Inside a core, sequence dim tiled with the layout
s = p*64 + j (p = SBUF partition, j = free) so every DMA is contiguous
per partition.  Softmax is computed without max subtraction (energies
are bounded by sum|V| ~ 16, exp cannot overflow in f32); exp partial
sums ride on the activation instruction's accum_out, the final
normalizer is replicated across partitions with an all-ones matmul,
and ctx is accumulated on the tensor engine in PSUM.
"""

import numpy as np

import concourse.bass as bass
import concourse.bacc as bacc
import concourse.tile as tile
from concourse import mybir
from contextlib import ExitStack

F32 = mybir.dt.float32
AX = mybir.AluOpType
AF = mybir.ActivationFunctionType

P = 128
FLOAT_MIN = -100000.0


def build_bahdanau(BP=2, S=8192, K=256, Q=1024, JC=8, energy_mode="stt",
                   masked=False, add_mode="pe", repeat=1, stage="full",
                   ctx_pe_j=4):
    """Build the per-core Bass program.

    BP: batches per core.  JC: j-columns per streamed tile (tile free dim
    JC*K).  energy_mode:
      "mulred": tensor_mul by V then one big tensor_reduce  (2 DVE passes)
      "stt":    per-j fused scalar_tensor_tensor with accum_out (1 DVE pass)
    masked: emit the kv_mask arithmetic (exact form m*e + (1-m)*FLOAT_MIN).
      When the mask is all-ones (the common case) build with masked=False:
      the mask is then mathematically a no-op and is skipped entirely.
    """
    JTOT = S // P          # j columns per partition per batch
    NT = JTOT // JC        # streamed tiles per batch
    KH = K // P            # k partition-halves of W
    QC = Q // P            # q chunks of W/query

    nc = bacc.Bacc("TRN2", target_bir_lowering=False, debug=False)

    keys_d = nc.dram_tensor("keys", [BP, S, K], F32, kind="ExternalInput")
    values_d = nc.dram_tensor("values", [BP, S, K], F32, kind="ExternalInput")
    mask_d = nc.dram_tensor("mask", [BP, S], F32, kind="ExternalInput")
    query_d = nc.dram_tensor("query", [BP, Q], F32, kind="ExternalInput")
    w_d = nc.dram_tensor("w_attr", [K, Q], F32, kind="ExternalInput")
    b_d = nc.dram_tensor("b_attr", [K], F32, kind="ExternalInput")
    v_d = nc.dram_tensor("v_attr", [K], F32, kind="ExternalInput")

    scores_d = nc.dram_tensor("scores_out", [BP, S], F32, kind="ExternalOutput")
    ctx_d = nc.dram_tensor("ctx_out", [BP, K], F32, kind="ExternalOutput")

    ident_d = nc.inline_tensor(np.eye(P, dtype=np.float32), name="ident")
    ones_d = nc.inline_tensor(np.ones((P, P), dtype=np.float32), name="ones_pp")

    keys_v = keys_d[:].rearrange("b (p j) k -> b p j k", p=P)
    values_v = values_d[:].rearrange("b (p j) k -> b p j k", p=P)
    mask_v = mask_d[:].rearrange("b (p j) -> b p j", p=P)
    scores_v = scores_d[:].rearrange("b (p j) -> b p j", p=P)
    w_v = w_d[:].rearrange("(h p) q -> p h q", p=P)

    with tile.TileContext(nc) as tc, ExitStack() as ctx:
        consts = ctx.enter_context(tc.tile_pool(name="consts", bufs=1))
        prol = ctx.enter_context(tc.tile_pool(name="prol", bufs=1))
        kp = ctx.enter_context(tc.tile_pool(name="keysp", bufs=7))
        vp = ctx.enter_context(tc.tile_pool(name="valsp", bufs=7))
        ttp = ctx.enter_context(tc.tile_pool(name="tanhp", bufs=4))
        mp = ctx.enter_context(tc.tile_pool(name="maskp", bufs=2))
        epool = ctx.enter_context(tc.tile_pool(name="energyp", bufs=3))
        sp = ctx.enter_context(tc.tile_pool(name="scoresp", bufs=2))
        zp = ctx.enter_context(tc.tile_pool(name="zpartp", bufs=2))
        outp = ctx.enter_context(tc.tile_pool(name="outsp", bufs=2))
        scr = ctx.enter_context(tc.tile_pool(name="scratch", bufs=2))

        pst = ctx.enter_context(tc.tile_pool(name="pst", bufs=2, space="PSUM"))
        psc = ctx.enter_context(tc.tile_pool(name="psc", bufs=2, space="PSUM"))
        if add_mode == "pe":
            psa = ctx.enter_context(tc.tile_pool(name="psa", bufs=2, space="PSUM"))

        # ---- constants ----
        ident_sb = consts.tile([P, P], F32)
        nc.sync.dma_start(out=ident_sb, in_=ident_d[:])
        ones_sb = consts.tile([P, P], F32)
        nc.sync.dma_start(out=ones_sb, in_=ones_d[:])
        if add_mode == "pe":
            ident_r = consts.tile([P, P], mybir.dt.float32r)
            nc.sync.dma_start(
                out=ident_r, in_=ident_d[:].bitcast(mybir.dt.float32r)
            )
            ones_r = consts.tile([1, P], mybir.dt.float32r)
            nc.sync.dma_start(
                out=ones_r, in_=ones_d[0:1, :].bitcast(mybir.dt.float32r)
            )
        v_sb = consts.tile([P, K], F32)
        nc.sync.dma_start(out=v_sb, in_=v_d[:].partition_broadcast(P))
        b1_sb = consts.tile([1, K], F32)
        nc.sync.dma_start(out=b1_sb, in_=b_d[:].partition_broadcast(1))

        # ---- prologue: q projection  q = query @ W.T + b ----
        w_sb = prol.tile([P, KH, Q], F32)
        nc.sync.dma_start(out=w_sb, in_=w_v)
        q_sb = prol.tile([BP, Q], F32)
        nc.sync.dma_start(out=q_sb, in_=query_d[:])

        # transpose W -> Wt [q-part, k-free], and query -> qT [q-part, b]
        wt_sb = prol.tile([P, QC, K], F32)
        qt_sb = prol.tile([P, QC, BP], F32)
        for c in range(QC):
            for h in range(KH):
                tp = pst.tile([P, P], F32, tag="tp")
                nc.tensor.transpose(
                    tp[:, :], w_sb[:, h, c * P:(c + 1) * P], ident_sb[:, :]
                )
                nc.scalar.copy(wt_sb[:, c, h * P:(h + 1) * P], tp[:, :])
            tq = pst.tile([P, P], F32, tag="tp")
            nc.tensor.transpose(
                tq[:, 0:BP], q_sb[:, c * P:(c + 1) * P], ident_sb[0:BP, 0:BP]
            )
            nc.scalar.copy(qt_sb[:, c, :], tq[:, 0:BP])

        # per-batch projection row (each lands at partition 0)
        qbc = []      # dve add-mode: qb broadcast [P, K] per batch
        qbrow_r = []  # pe add-mode: qb row repeated JC times, float32r
        for b in range(BP):
            qp_ps = pst.tile([P, 2 * K], F32, tag="tp")
            for c in range(QC):
                nc.tensor.matmul(
                    qp_ps[0:1, 0:K], qt_sb[:, c, b:b + 1], wt_sb[:, c, :],
                    start=(c == 0), stop=(c == QC - 1),
                )
            qb_row = prol.tile([1, K], F32, tag=f"qbrow{b}")
            nc.vector.tensor_add(qb_row[:, :], qp_ps[0:1, 0:K], b1_sb[:, :])

            if add_mode == "pe":
                qr = prol.tile([1, JC * K], mybir.dt.float32r, tag=f"qbr{b}")
                nc.vector.tensor_copy(
                    qr[:, :].rearrange("p (j k) -> p j k", j=JC),
                    qb_row[:, :]
                    .rearrange("p (j k) -> p j k", j=1)
                    .to_broadcast([1, JC, K]),
                )
                qbrow_r.append(qr)
            else:
                qbb = pst.tile([P, 2 * K], F32, tag="tp")
                nc.tensor.matmul(
                    qbb[:, 0:K], ones_sb[0:1, :], qb_row[:, :],
                    start=True, stop=True,
                )
                qb_b = prol.tile([P, K], F32, tag=f"qbc{b}")
                nc.scalar.copy(qb_b[:, :], qbb[:, 0:K])
                qbc.append(qb_b)

        # ---- main loop (optionally repeated on-device for benchmarking) ----
        rep_cm = (
            tc.For_i(0, repeat, 1,
                     hint_engines=(mybir.EngineType.PE, mybir.EngineType.DVE,
                                   mybir.EngineType.Activation,
                                   mybir.EngineType.SP))
            if repeat > 1 else None
        )
        if rep_cm is not None:
            rep_cm.__enter__()
        for b in range(BP):
            scores_sb = sp.tile([P, JTOT], F32, tag="scores")
            zpart = zp.tile([P, NT], F32, tag="zpart")
            ctx_ps = psc.tile([1, K], F32, tag="ctx")
            if ctx_pe_j < JC:
                # DVE-side ctx accumulator (partition-partial sums)
                acc_sb = outp.tile([P, K], F32, tag="acc")
                nc.vector.memset(acc_sb[:, :], 0.0)

            if add_mode != "pe":
                qb_bcast = (
                    qbc[b][:, :]
                    .rearrange("p (j k) -> p j k", j=1)
                    .to_broadcast([P, JC, K])
                )
            v_bcast = (
                v_sb[:, :]
                .rearrange("p (j k) -> p j k", j=1)
                .to_broadcast([P, JC, K])
            )

            for ji in range(NT):
                kdt = mybir.dt.float32r if add_mode == "pe" else F32
                kt = kp.tile([P, JC * K], kdt, tag="kt")
                nc.sync.dma_start(
                    out=kt,
                    in_=keys_v[b, :, ji * JC:(ji + 1) * JC, :].bitcast(kdt),
                )
                vt = vp.tile([P, JC, K], mybir.dt.float32r, tag="vt")
                nc.sync.dma_start(
                    out=vt,
                    in_=values_v[b, :, ji * JC:(ji + 1) * JC, :].bitcast(
                        mybir.dt.float32r
                    ),
                )
                if masked:
                    mt = mp.tile([P, JC], F32, tag="mt")
                    nc.sync.dma_start(
                        out=mt, in_=mask_v[b, :, ji * JC:(ji + 1) * JC]
                    )

                if add_mode == "pe":
                    # tt = tanh(kt + q + b): broadcast-add on PE (fp32r,
                    # ~1e-6 rounding), tanh reads PSUM.  High priority so
                    # adds/tanh sort ahead of earlier tiles' ctx matmuls in
                    # the in-order PE/ACT streams (ctx can drain late; the
                    # add chain gates everything downstream).
                    tt = ttp.tile([P, JC * K], F32, tag="tt")
                    HKE = 1024  # psum chunk (2 banks)
                    with tc.high_priority(offset=26):
                        for h in range(JC * K // HKE):
                            ps = psa.tile([P, HKE], F32, tag="addps")
                            for c in range(0, HKE, 512):
                                lo = h * HKE + c
                                nc.tensor.matmul(
                                    ps[:, c:c + 512], ones_r[:, :],
                                    qbrow_r[b][:, lo:lo + 512],
                                    start=True, stop=False,
                                )
                                nc.tensor.matmul(
                                    ps[:, c:c + 512], ident_r[:, :],
                                    kt[:, lo:lo + 512],
                                    start=False, stop=True,
                                )
                            nc.scalar.activation(
                                tt[:, h * HKE:(h + 1) * HKE], ps[:, :], AF.Tanh
                            )
                else:
                    tt = kt
                    with tc.high_priority(offset=24):
                        nc.vector.tensor_add(
                            kt[:, :].rearrange("p (j k) -> p j k", j=JC),
                            kt[:, :].rearrange("p (j k) -> p j k", j=JC),
                            qb_bcast,
                        )
                        nc.scalar.activation(kt[:, :], kt[:, :], AF.Tanh)

                if stage == "tanh":
                    continue
                # energies e[p, j] = sum_k tt * V
                et = epool.tile([P, JC], F32, tag="et")
                if energy_mode == "mulred":
                    mm = scr.tile([P, JC, K], F32, tag="mm")
                    nc.vector.tensor_mul(
                        mm[:, :, :],
                        tt[:, :].rearrange("p (j k) -> p j k", j=JC),
                        v_bcast,
                    )
                    nc.vector.tensor_reduce(
                        et[:, :], mm[:, :, :], axis=mybir.AxisListType.X, op=AX.add
                    )
                else:
                    junk = scr.tile([P, K], F32, tag="junk")
                    for jj in range(JC):
                        nc.vector.scalar_tensor_tensor(
                            out=junk[:, :],
                            in0=tt[:, jj * K:(jj + 1) * K],
                            scalar=1.0,
                            in1=v_sb[:, :],
                            op0=AX.mult,
                            op1=AX.mult,
                            accum_out=et[:, jj:jj + 1],
                        )

                if masked:
                    # exact masked energy: e' = m*e + (1-m)*FLOAT_MIN
                    # om = m*(-FLOAT_MIN) + FLOAT_MIN  (== (1-m)*FLOAT_MIN, 0 at m=1)
                    om = mp.tile([P, JC], F32, tag="om")
                    nc.vector.tensor_scalar(
                        out=om[:, :], in0=mt[:, :],
                        scalar1=-FLOAT_MIN, scalar2=FLOAT_MIN,
                        op0=AX.mult, op1=AX.add,
                    )
                    nc.vector.tensor_mul(et[:, :], et[:, :], mt[:, :])
                    nc.vector.tensor_add(et[:, :], et[:, :], om[:, :])

                # p = exp(e), accumulate partial Z per partition
                nc.scalar.activation(
                    scores_sb[:, ji * JC:(ji + 1) * JC], et[:, :], AF.Exp,
                    bias=0.0, scale=1.0,
                    accum_out=zpart[:, ji:ji + 1],
                )
                if stage == "energy":
                    continue
                # ctx accumulation, split between PE (matmul into PSUM) and
                # DVE (fused multiply-accumulate with the per-partition score
                # as the tensor_scalar operand).  The per-column fp32r weight
                # reload makes each PE matmul ~700ns on HW, so PE alone
                # becomes the bottleneck; DVE absorbs the rest.
                if ctx_pe_j > 0:
                    pcol = scr.tile([P, JC], mybir.dt.float32r, tag="pcol")
                    nc.vector.tensor_copy(
                        pcol[:, :], scores_sb[:, ji * JC:(ji + 1) * JC]
                    )
                with tc.high_priority(offset=-40):
                    # deprioritize: ctx accumulation only gates the batch
                    # epilogue, not the per-tile dataflow — let it drain late
                    for jj in range(JC):
                        col = ji * JC + jj
                        if jj < ctx_pe_j:
                            nc.tensor.matmul(
                                ctx_ps[:, :],
                                pcol[:, jj:jj + 1],
                                vt[:, jj, :],
                                start=(ji == 0 and jj == 0),
                                stop=(ctx_pe_j == JC and ji == NT - 1
                                      and jj == JC - 1),
                            )
                        else:
                            nc.vector.scalar_tensor_tensor(
                                out=acc_sb[:, :],
                                in0=vt[:, jj, :].bitcast(F32),
                                scalar=scores_sb[:, col:col + 1],
                                in1=acc_sb[:, :],
                                op0=AX.mult,
                                op1=AX.add,
                            )

            if stage != "full":
                continue
            # normalizer Z = sum over partitions+tiles, replicated to all
            # partitions.  High priority: these are tiny ops that the next
            # batch's work floods out of the engine streams otherwise,
            # delaying the output DMAs by tens of us.
            with tc.high_priority():
                zc = zp.tile([P, 1], F32, tag="zc")
                nc.vector.tensor_reduce(
                    zc[:, :], zpart[:, :], axis=mybir.AxisListType.X, op=AX.add
                )
                zrep = pst.tile([P, P], F32, tag="tp")
                nc.tensor.matmul(zrep[:, 0:1], ones_sb[:, :], zc[:, :],
                                 start=True, stop=True)
                rz = zp.tile([P, 1], F32, tag="rz")
                nc.vector.reciprocal(rz[:, :], zrep[:, 0:1])

                # normalize + store.  Output DMAs go through the Pool
                # sequencer: on the in-order SP stream they would stall the
                # next batch's input loads behind this batch's epilogue.
                nc.vector.tensor_scalar_mul(
                    scores_sb[:, :], scores_sb[:, :], rz[:, :]
                )
                nc.gpsimd.dma_start(out=scores_v[b], in_=scores_sb[:, :])

                # fold the DVE-side partition-partial ctx into the PSUM total
                if ctx_pe_j < JC:
                    nc.tensor.matmul(
                        ctx_ps[:, :], ones_sb[:, 0:1], acc_sb[:, :],
                        start=(ctx_pe_j == 0), stop=True,
                    )
                cs = outp.tile([1, K], F32, tag="cs")
                nc.vector.tensor_scalar_mul(cs[:, :], ctx_ps[:, :], rz[0:1, 0:1])
                nc.gpsimd.dma_start(out=ctx_d[b:b + 1, :], in_=cs[:, :])

        if rep_cm is not None:
            rep_cm.__exit__(None, None, None)

    nc.compile()
    return nc


_CACHE = {}


def _get_nc(masked):
    key = ("nc", masked)
    if key not in _CACHE:
        _CACHE[key] = build_bahdanau(masked=masked)
    return _CACHE[key]


def kernel(**inputs):
    from concourse.bass_utils import run_bass_kernel_spmd

    query = np.ascontiguousarray(np.asarray(inputs["query"], dtype=np.float32))
    keys = np.ascontiguousarray(np.asarray(inputs["keys"], dtype=np.float32))
    values = np.ascontiguousarray(np.asarray(inputs["values"], dtype=np.float32))
    kv_mask = np.ascontiguousarray(
        np.asarray(inputs["kv_mask"], dtype=np.float32)[:, :, 0]
    )
    w = np.ascontiguousarray(np.asarray(inputs["W_attr"], dtype=np.float32))
    bb = np.ascontiguousarray(np.asarray(inputs["b_attr"], dtype=np.float32))
    vv = np.ascontiguousarray(np.asarray(inputs["V_attr"], dtype=np.float32))

    B = keys.shape[0]
    NCORES = 8
    BP = B // NCORES

    masked = not bool(np.all(kv_mask == 1.0))
    nc = _get_nc(masked)
    in_maps = []
    for i in range(NCORES):
        sl = slice(i * BP, (i + 1) * BP)
        in_maps.append({
            "keys": np.ascontiguousarray(keys[sl]),
            "values": np.ascontiguousarray(values[sl]),
            "mask": np.ascontiguousarray(kv_mask[sl]),
            "query": np.ascontiguousarray(query[sl]),
            "w_attr": w,
            "b_attr": bb,
            "v_attr": vv,
        })

    res = run_bass_kernel_spmd(nc, in_maps, core_ids=list(range(NCORES)))
    scores = np.concatenate([r["scores_out"] for r in res.results], axis=0)
    ctxv = np.concatenate([r["ctx_out"] for r in res.results], axis=0)
    return scores[:, :, None].astype(np.float32), ctxv.astype(np.float32)


# revision 47
# speedup vs baseline: 5.1250x; 1.1021x over previous
"""Bahdanau attention Trainium2 kernel.

Problem (per full input):
    query   [16, 1024] f32
    keys    [16, 8192, 256] f32
    values  [16, 8192, 256] f32
    kv_mask [16, 8192, 1] f32
    W_attr  [256, 1024], b_attr [256], V_attr [256] f32

    q = query @ W_attr.T                       # [B, K]
    e = (tanh(keys + q + b) @ V)               # [B, S]
    e = mask*e + (1-mask)*(-1e5)
    scores = softmax(e, axis=S)                # [B, S, 1]
    ctx = sum_s values * scores                # [B, K]

Strategy: data-parallel over batch across 8 cores (2 batches/core);
params replicated.  Inside a core, sequence dim tiled with the layout
s = p*64 + j (p = SBUF partition, j = free) so every DMA is contiguous
per partition.  Softmax is computed without max subtraction (energies
are bounded by sum|V| ~ 16, exp cannot overflow in f32); exp partial
sums ride on the activation instruction's accum_out, the final
normalizer is replicated across partitions with an all-ones matmul,
and ctx is accumulated on the tensor engine in PSUM.
"""

import numpy as np

import concourse.bass as bass
import concourse.bacc as bacc
import concourse.tile as tile
from concourse import mybir
from contextlib import ExitStack

F32 = mybir.dt.float32
AX = mybir.AluOpType
AF = mybir.ActivationFunctionType

P = 128
FLOAT_MIN = -100000.0


def build_bahdanau(BP=2, S=8192, K=256, Q=1024, JC=8, energy_mode="stt",
                   masked=False, add_mode="pe", repeat=1, stage="full",
                   ctx_pe_j=4):
    """Build the per-core Bass program.

    BP: batches per core.  JC: j-columns per streamed tile (tile free dim
    JC*K).  energy_mode:
      "mulred": tensor_mul by V then one big tensor_reduce  (2 DVE passes)
      "stt":    per-j fused scalar_tensor_tensor with accum_out (1 DVE pass)
    masked: emit the kv_mask arithmetic (exact form m*e + (1-m)*FLOAT_MIN).
      When the mask is all-ones (the common case) build with masked=False:
      the mask is then mathematically a no-op and is skipped entirely.
    """
    JTOT = S // P          # j columns per partition per batch
    NT = JTOT // JC        # streamed tiles per batch
    KH = K // P            # k partition-halves of W
    QC = Q // P            # q chunks of W/query

    nc = bacc.Bacc("TRN2", target_bir_lowering=False, debug=False)

    keys_d = nc.dram_tensor("keys", [BP, S, K], F32, kind="ExternalInput")
    values_d = nc.dram_tensor("values", [BP, S, K], F32, kind="ExternalInput")
    mask_d = nc.dram_tensor("mask", [BP, S], F32, kind="ExternalInput")
    query_d = nc.dram_tensor("query", [BP, Q], F32, kind="ExternalInput")
    w_d = nc.dram_tensor("w_attr", [K, Q], F32, kind="ExternalInput")
    b_d = nc.dram_tensor("b_attr", [K], F32, kind="ExternalInput")
    v_d = nc.dram_tensor("v_attr", [K], F32, kind="ExternalInput")

    scores_d = nc.dram_tensor("scores_out", [BP, S], F32, kind="ExternalOutput")
    ctx_d = nc.dram_tensor("ctx_out", [BP, K], F32, kind="ExternalOutput")

    ident_d = nc.inline_tensor(np.eye(P, dtype=np.float32), name="ident")
    ones_d = nc.inline_tensor(np.ones((P, P), dtype=np.float32), name="ones_pp")

    keys_v = keys_d[:].rearrange("b (p j) k -> b p j k", p=P)
    values_v = values_d[:].rearrange("b (p j) k -> b p j k", p=P)
    mask_v = mask_d[:].rearrange("b (p j) -> b p j", p=P)
    scores_v = scores_d[:].rearrange("b (p j) -> b p j", p=P)
    w_v = w_d[:].rearrange("(h p) q -> p h q", p=P)

    with tile.TileContext(nc) as tc, ExitStack() as ctx:
        consts = ctx.enter_context(tc.tile_pool(name="consts", bufs=1))
        prol = ctx.enter_context(tc.tile_pool(name="prol", bufs=1))
        kp = ctx.enter_context(tc.tile_pool(name="keysp", bufs=7))
        vp = ctx.enter_context(tc.tile_pool(name="valsp", bufs=7))
        ttp = ctx.enter_context(tc.tile_pool(name="tanhp", bufs=4))
        mp = ctx.enter_context(tc.tile_pool(name="maskp", bufs=2))
        epool = ctx.enter_context(tc.tile_pool(name="energyp", bufs=3))
        sp = ctx.enter_context(tc.tile_pool(name="scoresp", bufs=2))
        zp = ctx.enter_context(tc.tile_pool(name="zpartp", bufs=2))
        outp = ctx.enter_context(tc.tile_pool(name="outsp", bufs=2))
        scr = ctx.enter_context(tc.tile_pool(name="scratch", bufs=2))

        pst = ctx.enter_context(tc.tile_pool(name="pst", bufs=2, space="PSUM"))
        psc = ctx.enter_context(tc.tile_pool(name="psc", bufs=2, space="PSUM"))
        if add_mode == "pe":
            psa = ctx.enter_context(tc.tile_pool(name="psa", bufs=2, space="PSUM"))

        # ---- constants ----
        ident_sb = consts.tile([P, P], F32)
        nc.sync.dma_start(out=ident_sb, in_=ident_d[:])
        ones_sb = consts.tile([P, P], F32)
        nc.sync.dma_start(out=ones_sb, in_=ones_d[:])
        if add_mode == "pe":
            ident_r = consts.tile([P, P], mybir.dt.float32r)
            nc.sync.dma_start(
                out=ident_r, in_=ident_d[:].bitcast(mybir.dt.float32r)
            )
            ones_r = consts.tile([1, P], mybir.dt.float32r)
            nc.sync.dma_start(
                out=ones_r, in_=ones_d[0:1, :].bitcast(mybir.dt.float32r)
            )
        v_sb = consts.tile([P, K], F32)
        nc.sync.dma_start(out=v_sb, in_=v_d[:].partition_broadcast(P))
        b1_sb = consts.tile([1, K], F32)
        nc.sync.dma_start(out=b1_sb, in_=b_d[:].partition_broadcast(1))

        # ---- prologue: q projection  q = query @ W.T + b ----
        w_sb = prol.tile([P, KH, Q], F32)
        nc.sync.dma_start(out=w_sb, in_=w_v)
        q_sb = prol.tile([BP, Q], F32)
        nc.sync.dma_start(out=q_sb, in_=query_d[:])

        # transpose W -> Wt [q-part, k-free], and query -> qT [q-part, b]
        wt_sb = prol.tile([P, QC, K], F32)
        qt_sb = prol.tile([P, QC, BP], F32)
        for c in range(QC):
            for h in range(KH):
                tp = pst.tile([P, P], F32, tag="tp")
                nc.tensor.transpose(
                    tp[:, :], w_sb[:, h, c * P:(c + 1) * P], ident_sb[:, :]
                )
                nc.scalar.copy(wt_sb[:, c, h * P:(h + 1) * P], tp[:, :])
            tq = pst.tile([P, P], F32, tag="tp")
            nc.tensor.transpose(
                tq[:, 0:BP], q_sb[:, c * P:(c + 1) * P], ident_sb[0:BP, 0:BP]
            )
            nc.scalar.copy(qt_sb[:, c, :], tq[:, 0:BP])

        # per-batch projection row (each lands at partition 0)
        qbc = []      # dve add-mode: qb broadcast [P, K] per batch
        qbrow_r = []  # pe add-mode: qb row repeated JC times, float32r
        for b in range(BP):
            qp_ps = pst.tile([P, 2 * K], F32, tag="tp")
            for c in range(QC):
                nc.tensor.matmul(
                    qp_ps[0:1, 0:K], qt_sb[:, c, b:b + 1], wt_sb[:, c, :],
                    start=(c == 0), stop=(c == QC - 1),
                )
            qb_row = prol.tile([1, K], F32, tag=f"qbrow{b}")
            nc.vector.tensor_add(qb_row[:, :], qp_ps[0:1, 0:K], b1_sb[:, :])

            if add_mode == "pe":
                qr = prol.tile([1, JC * K], mybir.dt.float32r, tag=f"qbr{b}")
                nc.vector.tensor_copy(
                    qr[:, :].rearrange("p (j k) -> p j k", j=JC),
                    qb_row[:, :]
                    .rearrange("p (j k) -> p j k", j=1)
                    .to_broadcast([1, JC, K]),
                )
                qbrow_r.append(qr)
            else:
                qbb = pst.tile([P, 2 * K], F32, tag="tp")
                nc.tensor.matmul(
                    qbb[:, 0:K], ones_sb[0:1, :], qb_row[:, :],
                    start=True, stop=True,
                )
                qb_b = prol.tile([P, K], F32, tag=f"qbc{b}")
                nc.scalar.copy(qb_b[:, :], qbb[:, 0:K])
                qbc.append(qb_b)

        # ---- main loop (optionally repeated on-device for benchmarking) ----
        rep_cm = (
            tc.For_i(0, repeat, 1,
                     hint_engines=(mybir.EngineType.PE, mybir.EngineType.DVE,
                                   mybir.EngineType.Activation,
                                   mybir.EngineType.SP))
            if repeat > 1 else None
        )
        if rep_cm is not None:
            rep_cm.__enter__()
        for b in range(BP):
            scores_sb = sp.tile([P, JTOT], F32, tag="scores")
            zpart = zp.tile([P, NT], F32, tag="zpart")
            ctx_ps = psc.tile([1, K], F32, tag="ctx")
            if ctx_pe_j < JC:
                # DVE-side ctx accumulator (partition-partial sums)
                acc_sb = outp.tile([P, K], F32, tag="acc")
                nc.vector.memset(acc_sb[:, :], 0.0)

            if add_mode != "pe":
                qb_bcast = (
                    qbc[b][:, :]
                    .rearrange("p (j k) -> p j k", j=1)
                    .to_broadcast([P, JC, K])
                )
            v_bcast = (
                v_sb[:, :]
                .rearrange("p (j k) -> p j k", j=1)
                .to_broadcast([P, JC, K])
            )

            for ji in range(NT):
                kdt = mybir.dt.float32r if add_mode == "pe" else F32
                kt = kp.tile([P, JC * K], kdt, tag="kt")
                nc.sync.dma_start(
                    out=kt,
                    in_=keys_v[b, :, ji * JC:(ji + 1) * JC, :].bitcast(kdt),
                )
                vt = vp.tile([P, JC, K], mybir.dt.float32r, tag="vt")
                nc.sync.dma_start(
                    out=vt,
                    in_=values_v[b, :, ji * JC:(ji + 1) * JC, :].bitcast(
                        mybir.dt.float32r
                    ),
                )
                if masked:
                    mt = mp.tile([P, JC], F32, tag="mt")
                    nc.sync.dma_start(
                        out=mt, in_=mask_v[b, :, ji * JC:(ji + 1) * JC]
                    )

                if add_mode == "pe":
                    # tt = tanh(kt + q + b): broadcast-add on PE (fp32r,
                    # ~1e-6 rounding), tanh reads PSUM.  High priority so
                    # adds/tanh sort ahead of earlier tiles' ctx matmuls in
                    # the in-order PE/ACT streams (ctx can drain late; the
                    # add chain gates everything downstream).
                    tt = ttp.tile([P, JC * K], F32, tag="tt")
                    HKE = 1024  # psum chunk (2 banks)
                    with tc.high_priority(offset=26):
                        for h in range(JC * K // HKE):
                            ps = psa.tile([P, HKE], F32, tag="addps")
                            for c in range(0, HKE, 512):
                                lo = h * HKE + c
                                nc.tensor.matmul(
                                    ps[:, c:c + 512], ones_r[:, :],
                                    qbrow_r[b][:, lo:lo + 512],
                                    start=True, stop=False,
                                )
                                nc.tensor.matmul(
                                    ps[:, c:c + 512], ident_r[:, :],
                                    kt[:, lo:lo + 512],
                                    start=False, stop=True,
                                )
                            nc.scalar.activation(
                                tt[:, h * HKE:(h + 1) * HKE], ps[:, :], AF.Tanh
                            )
                else:
                    tt = kt
                    with tc.high_priority(offset=24):
                        nc.vector.tensor_add(
                            kt[:, :].rearrange("p (j k) -> p j k", j=JC),
                            kt[:, :].rearrange("p (j k) -> p j k", j=JC),
                            qb_bcast,
                        )
                        nc.scalar.activation(kt[:, :], kt[:, :], AF.Tanh)

                if stage == "tanh":
                    continue
                # energies e[p, j] = sum_k tt * V
                et = epool.tile([P, JC], F32, tag="et")
                if energy_mode == "mulred":
                    mm = scr.tile([P, JC, K], F32, tag="mm")
                    nc.vector.tensor_mul(
                        mm[:, :, :],
                        tt[:, :].rearrange("p (j k) -> p j k", j=JC),
                        v_bcast,
                    )
                    nc.vector.tensor_reduce(
                        et[:, :], mm[:, :, :], axis=mybir.AxisListType.X, op=AX.add
                    )
                else:
                    junk = scr.tile([P, K], F32, tag="junk")
                    for jj in range(JC):
                        nc.vector.scalar_tensor_tensor(
                            out=junk[:, :],
                            in0=tt[:, jj * K:(jj + 1) * K],
                            scalar=1.0,
                            in1=v_sb[:, :],
                            op0=AX.mult,
                            op1=AX.mult,
                            accum_out=et[:, jj:jj + 1],
                        )

                if masked:
                    # exact masked energy: e' = m*e + (1-m)*FLOAT_MIN
                    # om = m*(-FLOAT_MIN) + FLOAT_MIN  (== (1-m)*FLOAT_MIN, 0 at m=1)
                    om = mp.tile([P, JC], F32, tag="om")
                    nc.vector.tensor_scalar(
                        out=om[:, :], in0=mt[:, :],
                        scalar1=-FLOAT_MIN, scalar2=FLOAT_MIN,
                        op0=AX.mult, op1=AX.add,
                    )
                    nc.vector.tensor_mul(et[:, :], et[:, :], mt[:, :])
                    nc.vector.tensor_add(et[:, :], et[:, :], om[:, :])

                # p = exp(e), accumulate partial Z per partition
                nc.scalar.activation(
                    scores_sb[:, ji * JC:(ji + 1) * JC], et[:, :], AF.Exp,
                    bias=0.0, scale=1.0,
                    accum_out=zpart[:, ji:ji + 1],
                )
                if stage == "energy":
                    continue
                # ctx accumulation, split between PE (matmul into PSUM) and
                # DVE (fused multiply-accumulate with the per-partition score
                # as the tensor_scalar operand).  The per-column fp32r weight
                # reload makes each PE matmul ~700ns on HW, so PE alone
                # becomes the bottleneck; DVE absorbs the rest.
                if ctx_pe_j > 0:
                    pcol = scr.tile([P, JC], mybir.dt.float32r, tag="pcol")
                    nc.vector.tensor_copy(
                        pcol[:, :], scores_sb[:, ji * JC:(ji + 1) * JC]
                    )
                with tc.high_priority(offset=-40):
                    # deprioritize: ctx accumulation only gates the batch
                    # epilogue, not the per-tile dataflow — let it drain late
                    for jj in range(JC):
                        col = ji * JC + jj
                        if jj < ctx_pe_j:
                            nc.tensor.matmul(
                                ctx_ps[:, :],
                                pcol[:, jj:jj + 1],
                                vt[:, jj, :],
                                start=(ji == 0 and jj == 0),
                                stop=(ctx_pe_j == JC and ji == NT - 1
                                      and jj == JC - 1),
                            )
                        else:
                            nc.vector.scalar_tensor_tensor(
                                out=acc_sb[:, :],
                                in0=vt[:, jj, :].bitcast(F32),
                                scalar=scores_sb[:, col:col + 1],
                                in1=acc_sb[:, :],
                                op0=AX.mult,
                                op1=AX.add,
                            )

            if stage != "full":
                continue
            # normalizer Z = sum over partitions+tiles, replicated to all
            # partitions.  High priority: these are tiny ops that the next
            # batch's work floods out of the engine streams otherwise,
            # delaying the output DMAs by tens of us.
            with tc.high_priority():
                zc = zp.tile([P, 1], F32, tag="zc")
                nc.vector.tensor_reduce(
                    zc[:, :], zpart[:, :], axis=mybir.AxisListType.X, op=AX.add
                )
                zrep = pst.tile([P, P], F32, tag="tp")
                nc.tensor.matmul(zrep[:, 0:1], ones_sb[:, :], zc[:, :],
                                 start=True, stop=True)
                rz = zp.tile([P, 1], F32, tag="rz")
                nc.vector.reciprocal(rz[:, :], zrep[:, 0:1])

                # normalize + store.  Output DMAs go through the Pool
                # sequencer: on the in-order SP stream they would stall the
                # next batch's input loads behind this batch's epilogue.
                nc.vector.tensor_scalar_mul(
                    scores_sb[:, :], scores_sb[:, :], rz[:, :]
                )
                nc.gpsimd.dma_start(out=scores_v[b], in_=scores_sb[:, :])

                # fold the DVE-side partition-partial ctx into the PSUM total
                if ctx_pe_j < JC:
                    nc.tensor.matmul(
                        ctx_ps[:, :], ones_sb[:, 0:1], acc_sb[:, :],
                        start=(ctx_pe_j == 0), stop=True,
                    )
                cs = outp.tile([1, K], F32, tag="cs")
                nc.vector.tensor_scalar_mul(cs[:, :], ctx_ps[:, :], rz[0:1, 0:1])
                nc.gpsimd.dma_start(out=ctx_d[b:b + 1, :], in_=cs[:, :])

        if rep_cm is not None:
            rep_cm.__exit__(None, None, None)

    nc.compile()
    return nc


_CACHE = {}


def _get_nc(masked):
    key = ("nc", masked)
    if key not in _CACHE:
        _CACHE[key] = build_bahdanau(masked=masked)
    return _CACHE[key]


def kernel(**inputs):
    from concourse.bass_utils import run_bass_kernel_spmd

    query = np.ascontiguousarray(np.asarray(inputs["query"], dtype=np.float32))
    keys = np.ascontiguousarray(np.asarray(inputs["keys"], dtype=np.float32))
    values = np.ascontiguousarray(np.asarray(inputs["values"], dtype=np.float32))
    kv_mask = np.ascontiguousarray(
        np.asarray(inputs["kv_mask"], dtype=np.float32)[:, :, 0]
    )
    w = np.ascontiguousarray(np.asarray(inputs["W_attr"], dtype=np.float32))
    bb = np.ascontiguousarray(np.asarray(inputs["b_attr"], dtype=np.float32))
    vv = np.ascontiguousarray(np.asarray(inputs["V_attr"], dtype=np.float32))

    B = keys.shape[0]
    NCORES = 8
    BP = B // NCORES

    masked = not bool(np.all(kv_mask == 1.0))
    nc = _get_nc(masked)
    in_maps = []
    for i in range(NCORES):
        sl = slice(i * BP, (i + 1) * BP)
        in_maps.append({
            "keys": np.ascontiguousarray(keys[sl]),
            "values": np.ascontiguousarray(values[sl]),
            "mask": np.ascontiguousarray(kv_mask[sl]),
            "query": np.ascontiguousarray(query[sl]),
            "w_attr": w,
            "b_attr": bb,
            "v_attr": vv,
        })

    res = run_bass_kernel_spmd(nc, in_maps, core_ids=list(range(NCORES)))
    scores = np.concatenate([r["scores_out"] for r in res.results], axis=0)
    ctxv = np.concatenate([r["ctx_out"] for r in res.results], axis=0)
    return scores[:, :, None].astype(np.float32), ctxv.astype(np.float32)
